# revision 1
# baseline (speedup 1.0000x reference)
"""Trainium2 Bass kernel for nn_LocalAttention (sparse point-cloud attention).

Self-contained: accepts FULL unsharded inputs, shards across 8 NeuronCores
internally, returns the FULL (50000, 256) float32 output.

Distribution strategy: points (N) are sharded across the 8 cores in 6272-row
slices (128-aligned).  Neighbor indices are random over all N, so every core
builds a full neighbor-feature table [K(x) | V(x) | pos] in its own HBM
(phase A), then runs attention for its own shard, fetching neighbor rows with
indirect-DMA gathers (phase B).  Small projection weights are replicated.

Math folds (exact, done on host):
  - softmax(l + c) == softmax(l)  -> bk drops out of the logits entirely.
  - sum_k attn = 1                -> bv contributes bv @ Wo, folded into bo.
  - logits scale 1/sqrt(D)        -> folded into Wq, bq.
  - b1 folded into the rel-MLP via a ones-row appended to the transposed rel.
"""

import sys

import numpy as np

for _p in ("/opt/trn_rl_repo",):
    if _p not in sys.path:
        sys.path.insert(0, _p)

import ml_dtypes

import concourse.bass as bass
import concourse.tile as tile
from concourse import bacc, mybir
from concourse.bass import IndirectOffsetOnAxis
from concourse.masks import make_identity

BF16 = mybir.dt.bfloat16
F32 = mybir.dt.float32
I32 = mybir.dt.int32

N, C, H, D, K = 50000, 256, 8, 8 * 4, 16
assert C == H * D
SCALE = D**-0.5
EPS = 1e-5
NCORES = 8
P = 128
ROW = 520  # table row: 256 k | 256 v | 6 pos-bits | 2 pad  (bf16 elems)

nbf16 = ml_dtypes.bfloat16


def full_cfg():
    SH = 6272  # points per core (49 * 128); last core only 6096 valid
    return dict(NPAD=NCORES * SH, SH=SH, T=SH // P, G=NCORES * SH // P,
                ncores=NCORES)


# ---------------------------------------------------------------- host prep


def host_prep(inputs, cfg):
    """Full np inputs -> per-core in_maps for the SPMD kernel."""
    NPAD, SH, T, G = cfg["NPAD"], cfg["SH"], cfg["T"], cfg["G"]
    ncores = cfg["ncores"]
    n = inputs["x"].shape[0]

    x = np.asarray(inputs["x"], np.float32)
    pos = np.asarray(inputs["pos"], np.float32)
    idx = np.asarray(inputs["idx"]).astype(np.int32)
    Wq = np.asarray(inputs["Wq"], np.float32)
    bq = np.asarray(inputs["bq"], np.float32)
    Wk = np.asarray(inputs["Wk"], np.float32)
    Wv = np.asarray(inputs["Wv"], np.float32)
    Wo = np.asarray(inputs["Wo"], np.float32)
    bo = np.asarray(inputs["bo"], np.float32)
    bv = np.asarray(inputs["bv"], np.float32)
    W1 = np.asarray(inputs["W1"], np.float32)
    b1 = np.asarray(inputs["b1"], np.float32)
    W2 = np.asarray(inputs["W2"], np.float32)
    b2 = np.asarray(inputs["b2"], np.float32)
    gamma = np.asarray(inputs["gamma"], np.float32)
    beta = np.asarray(inputs["beta"], np.float32)

    xpad = np.zeros((NPAD, C), np.float32)
    xpad[:n] = x
    pospad = np.zeros((NPAD, 3), np.float32)
    pospad[:n] = pos
    idxpad = np.zeros((NPAD, K), np.int32)
    idxpad[:n] = idx

    # x^T tiles for PE lhsT: [g, p, j, m] = xpad[g*128 + m, j*128 + p]
    a = xpad.reshape(G, P, 2, P)  # [g, m, j, p]
    xtt = np.ascontiguousarray(a.transpose(0, 3, 2, 1)).astype(nbf16)

    # raw f32 pos bits as bf16 pairs, padded to 8 cols
    pbits = np.zeros((NPAD, 8), np.uint16)
    pbits[:, :6] = pospad.view(np.uint16).reshape(NPAD, 6)
    pbits = pbits.view(nbf16).reshape(G, P, 8)

    # weights: (128, 2, 256) layout -> w[c, j, o] = W[j*128 + c, o]
    def wlay(W):
        return np.ascontiguousarray(
            W.reshape(2, P, C).transpose(1, 0, 2)).astype(nbf16)

    wq_l = wlay(Wq * SCALE)
    wk_l = wlay(Wk)
    wv_l = wlay(Wv)
    wo_l = wlay(Wo)
    # rel-MLP as block-diagonal matmuls over (i|1, k) -> (k, j) rows
    w1aug = np.concatenate([W1, b1[None, :]], 0).astype(np.float32)  # (4, 64)
    w1big = np.zeros((4 * K, K * 64), np.float32)  # rows (i, k), cols (k, j)
    for k in range(K):
        for i in range(4):
            w1big[i * K + k, k * 64:(k + 1) * 64] = w1aug[i]
    # hidT block rows (kl, j) kl in {0,1} -> cols (kl, h)
    w2big = np.zeros((P, 2 * H), np.float32)
    for kl in range(2):
        w2big[kl * 64:(kl + 1) * 64, kl * H:(kl + 1) * H] = W2
    w2big = w2big.astype(nbf16)
    bq_s = (bq * SCALE).astype(np.float32)
    bo_eff = (bv @ Wo + bo).astype(np.float32)

    shared = dict(xtt=xtt, posbits=pbits, wq=wq_l, wk=wk_l, wv=wv_l,
                  wo=wo_l, w1b=w1big, w2=w2big, bq=bq_s, boeff=bo_eff,
                  b2=b2, gamma=gamma, beta=beta)

    in_maps = []
    for c in range(ncores):
        lo = c * SH
        sl = slice(lo, lo + SH)
        idxr = np.ascontiguousarray(
            idxpad[sl].reshape(T, P, K).transpose(1, 0, 2)).reshape(P, T * K)
        posr = np.ascontiguousarray(
            pospad[sl].reshape(T, P, 3).transpose(1, 0, 2)).reshape(P, T * 3)
        m = dict(shared)
        m.update(xres=np.ascontiguousarray(xpad[sl]),
                 xqt=np.ascontiguousarray(xtt[c * T:(c + 1) * T]),
                 idxr=idxr, posr=posr)
        in_maps.append(m)
    return in_maps


# ------------------------------------------------------------- bass program


def build_nc(cfg):
    NPAD, SH, T, G = cfg["NPAD"], cfg["SH"], cfg["T"], cfg["G"]

    nc = bacc.Bacc(trn_type="TRN2")

    # I/O
    xtt = nc.dram_tensor("xtt", [G, P, 2, P], BF16, kind="ExternalInput")
    posbits = nc.dram_tensor("posbits", [G, P, 8], BF16, kind="ExternalInput")
    xqt = nc.dram_tensor("xqt", [T, P, 2, P], BF16, kind="ExternalInput")
    xres = nc.dram_tensor("xres", [SH, C], F32, kind="ExternalInput")
    idxr = nc.dram_tensor("idxr", [P, T * K], I32, kind="ExternalInput")
    posr = nc.dram_tensor("posr", [P, T * 3], F32, kind="ExternalInput")
    wq = nc.dram_tensor("wq", [P, 2, C], BF16, kind="ExternalInput")
    wk = nc.dram_tensor("wk", [P, 2, C], BF16, kind="ExternalInput")
    wv = nc.dram_tensor("wv", [P, 2, C], BF16, kind="ExternalInput")
    wo = nc.dram_tensor("wo", [P, 2, C], BF16, kind="ExternalInput")
    w1b = nc.dram_tensor("w1b", [4 * K, K * 64], F32, kind="ExternalInput")
    w2 = nc.dram_tensor("w2", [P, 2 * H], BF16, kind="ExternalInput")
    bq = nc.dram_tensor("bq", [C], F32, kind="ExternalInput")
    boeff = nc.dram_tensor("boeff", [C], F32, kind="ExternalInput")
    b2 = nc.dram_tensor("b2", [H], F32, kind="ExternalInput")
    gamma = nc.dram_tensor("gamma", [C], F32, kind="ExternalInput")
    beta = nc.dram_tensor("beta", [C], F32, kind="ExternalInput")
    y = nc.dram_tensor("y", [SH, C], F32, kind="ExternalOutput")

    table = nc.dram_tensor("table", [NPAD, ROW], BF16)

    dbg = cfg.get("dbg")
    if dbg:
        dbg_table = nc.dram_tensor("dbg_table", [NPAD, ROW], BF16,
                                   kind="ExternalOutput")
        dbg_kv = nc.dram_tensor("dbg_kv", [P, K * ROW], BF16,
                                kind="ExternalOutput")
        dbg_q = nc.dram_tensor("dbg_q", [P, C], BF16, kind="ExternalOutput")
        dbg_log = nc.dram_tensor("dbg_log", [P, K * H], F32,
                                 kind="ExternalOutput")
        dbg_attn = nc.dram_tensor("dbg_attn", [P, K * H], BF16,
                                  kind="ExternalOutput")
        dbg_av = nc.dram_tensor("dbg_av", [P, C], BF16, kind="ExternalOutput")
        dbg_hid = nc.dram_tensor("dbg_hid", [P, K * 64], BF16,
                                 kind="ExternalOutput")
        dbg_rel = nc.dram_tensor("dbg_rel", [P, 4 * K], F32,
                                 kind="ExternalOutput")

    def bcast_vec(v, cols):
        # DRAM vector -> (128, cols) SBUF replicated across partitions
        return bass.AP(tensor=v.ap().tensor, offset=0, ap=[[0, P], [1, cols]])

    with tile.TileContext(nc) as tc:
        import contextlib

        with contextlib.ExitStack() as ctx:
            consts = ctx.enter_context(tc.tile_pool(name="consts", bufs=1))

            wq_sb = consts.tile([P, 2, C], BF16)
            wk_sb = consts.tile([P, 2, C], BF16)
            wv_sb = consts.tile([P, 2, C], BF16)
            wo_sb = consts.tile([P, 2, C], BF16)
            for t_sb, t_dr in ((wq_sb, wq), (wk_sb, wk), (wv_sb, wv),
                               (wo_sb, wo)):
                nc.sync.dma_start(out=t_sb[:], in_=t_dr[:, :, :])
            w1_sb = consts.tile([4 * K, K * 64], F32)
            nc.sync.dma_start(out=w1_sb[:], in_=w1b[:, :])
            w2_sb = consts.tile([P, 2 * H], BF16)
            nc.sync.dma_start(out=w2_sb[:], in_=w2[:, :])
            idx_sb = consts.tile([P, T * K], I32)
            nc.sync.dma_start(out=idx_sb[:], in_=idxr[:, :])
            posr_sb = consts.tile([P, T * 3], F32)
            nc.sync.dma_start(out=posr_sb[:], in_=posr[:, :])
            bq_sb = consts.tile([P, C], F32)
            nc.sync.dma_start(out=bq_sb[:], in_=bcast_vec(bq, C))
            bo_sb = consts.tile([P, C], F32)
            nc.sync.dma_start(out=bo_sb[:], in_=bcast_vec(boeff, C))
            b2_sb = consts.tile([P, H], F32)
            nc.sync.dma_start(out=b2_sb[:], in_=bcast_vec(b2, H))
            gam_sb = consts.tile([P, C], F32)
            nc.sync.dma_start(out=gam_sb[:], in_=bcast_vec(gamma, C))
            bet_sb = consts.tile([P, C], F32)
            nc.sync.dma_start(out=bet_sb[:], in_=bcast_vec(beta, C))
            eps_sb = consts.tile([P, 1], F32)
            nc.vector.memset(eps_sb[:], EPS)
            ident = consts.tile([P, P], F32)
            make_identity(nc, ident[:])
            ident_bf = consts.tile([P, P], BF16)
            nc.vector.tensor_copy(out=ident_bf[:], in_=ident[:])

            # ---------------- phase A: neighbor table [k | v | posbits]
            with tc.tile_pool(name="pa", bufs=4) as pa, \
                    tc.tile_pool(name="paps", bufs=2, space="PSUM") as paps:
                for g in range(G):
                    xt = pa.tile([P, 2, P], BF16, tag="xt")
                    nc.sync.dma_start(out=xt[:], in_=xtt[g, :, :, :])
                    row = pa.tile([P, ROW], BF16, tag="row")
                    kps = paps.tile([P, C], F32, tag="kps")
                    vps = paps.tile([P, C], F32, tag="vps")
                    for j in range(2):
                        nc.tensor.matmul(kps[:], lhsT=xt[:, j, :],
                                         rhs=wk_sb[:, j, :],
                                         start=(j == 0), stop=(j == 1))
                    for j in range(2):
                        nc.tensor.matmul(vps[:], lhsT=xt[:, j, :],
                                         rhs=wv_sb[:, j, :],
                                         start=(j == 0), stop=(j == 1))
                    nc.scalar.copy(out=row[:, 0:C], in_=kps[:])
                    nc.scalar.copy(out=row[:, C:2 * C], in_=vps[:])
                    nc.sync.dma_start(out=row[:, 2 * C:ROW],
                                      in_=posbits[g, :, :])
                    nc.sync.dma_start(out=table[g * P:(g + 1) * P, :],
                                      in_=row[:])
                    if dbg:
                        nc.sync.dma_start(
                            out=dbg_table[g * P:(g + 1) * P, :], in_=row[:])

            # ---------------- phase B: local attention on own shard
            with contextlib.ExitStack() as bctx:
                pb = bctx.enter_context(tc.tile_pool(name="pb", bufs=2))
                pkv = bctx.enter_context(tc.tile_pool(name="pkv", bufs=2))
                psm = bctx.enter_context(tc.tile_pool(name="psm", bufs=2))
                ps_mm = bctx.enter_context(
                    tc.tile_pool(name="ps_mm", bufs=2, space="PSUM"))
                ps_tp = bctx.enter_context(
                    tc.tile_pool(name="ps_tp", bufs=2, space="PSUM"))
                ps_h = bctx.enter_context(
                    tc.tile_pool(name="ps_h", bufs=1, space="PSUM"))
                ps_b = bctx.enter_context(
                    tc.tile_pool(name="ps_b", bufs=2, space="PSUM"))

                for t in range(T):
                    # --- Q projection (pre-scaled)
                    xq = pb.tile([P, 2, P], BF16, tag="xq")
                    nc.sync.dma_start(out=xq[:], in_=xqt[t, :, :, :])
                    qps = ps_mm.tile([P, C], F32, tag="mm")
                    for j in range(2):
                        nc.tensor.matmul(qps[:], lhsT=xq[:, j, :],
                                         rhs=wq_sb[:, j, :],
                                         start=(j == 0), stop=(j == 1))
                    q_sb = pb.tile([P, C], BF16, tag="q")
                    nc.vector.tensor_tensor(out=q_sb[:], in0=qps[:],
                                            in1=bq_sb[:],
                                            op=mybir.AluOpType.add)

                    # --- gather neighbor rows
    # HW contract for indirect gather: one row per partition per instruction
    # (≤128 indices), so fetch one neighbor slot per call.
                    kv = pkv.tile([P, K, ROW], BF16, tag="kv")
                    for k in range(K):
                        nc.gpsimd.indirect_dma_start(
                            out=kv[:, k, :], out_offset=None,
                            in_=table[:, :],
                            in_offset=IndirectOffsetOnAxis(
                                ap=idx_sb[:, t * K + k:t * K + k + 1],
                                axis=0))

                    # --- rel = pos_nb - pos  (plus ones row for b1)
                    rel = pb.tile([P, 4, K], F32, tag="rel")
                    nc.vector.memset(rel[:, 3, :], 1.0)
                    pos_nb = kv[:, :, 2 * C:2 * C + 6].bitcast(F32)
                    pnb = bass.AP(tensor=pos_nb.tensor, offset=pos_nb.offset,
                                  ap=[pos_nb.ap[0], [1, 3],
                                      [ROW // 2, K]])
                    psl = posr_sb[:, t * 3:(t + 1) * 3]
                    pbr = bass.AP(tensor=psl.tensor, offset=psl.offset,
                                  ap=[psl.ap[0], [1, 3], [0, K]])
                    nc.vector.tensor_tensor(out=rel[:, 0:3, :], in0=pnb,
                                            in1=pbr,
                                            op=mybir.AluOpType.subtract)

                    # --- rel MLP: hid = gelu([rel|1] @ [W1;b1])
                    relT_ps = ps_tp.tile([64, P], F32, tag="tp")
                    rel2d = rel[:].rearrange("p a k -> p (a k)")
                    nc.tensor.transpose(out=relT_ps[:], in_=rel2d,
                                        identity=ident[:])
                    relT = pb.tile([64, P], F32, tag="relT")
                    nc.scalar.copy(out=relT[:], in_=relT_ps[:])
                    hid_ps = ps_h.tile([P, K * 64], F32, tag="hid")
                    for j in range(2):
                        nc.tensor.matmul(hid_ps[:, j * 512:(j + 1) * 512],
                                         lhsT=relT[:],
                                         rhs=w1_sb[:, j * 512:(j + 1) * 512],
                                         start=True, stop=True)
                    hid_sb = pb.tile([P, K * 64], BF16, tag="hid_sb")
                    nc.scalar.activation(out=hid_sb[:], in_=hid_ps[:],
                                         func=mybir.ActivationFunctionType.Gelu)
                    hidT_ps = ps_tp.tile([P, 8, P], BF16, tag="tp")
                    for b in range(8):
                        nc.tensor.transpose(out=hidT_ps[:, b, :],
                                            in_=hid_sb[:, b * P:(b + 1) * P],
                                            identity=ident_bf[:])
                    hidT = pb.tile([P, 8, P], BF16, tag="hidT")
                    nc.scalar.copy(out=hidT[:], in_=hidT_ps[:])
                    bias_ps = ps_b.tile([P, K, H], F32, tag="bias")
                    for b in range(8):
                        nc.tensor.matmul(
                            bias_ps[:, 2 * b:2 * b + 2, :].rearrange(
                                "p a h -> p (a h)"),
                            lhsT=hidT[:, b, :], rhs=w2_sb[:],
                            start=True, stop=True)

                    # --- logits = q . k_nb  (+ mlp bias + b2), layout (k, h)
                    prod = pkv.tile([P, K, H, D], BF16, tag="prod")
                    kf = bass.AP(tensor=kv.tensor, offset=kv[:].offset,
                                 ap=[kv[:].ap[0], [ROW, K], [D, H], [1, D]])
                    qb = bass.AP(tensor=q_sb.tensor, offset=q_sb[:].offset,
                                 ap=[q_sb[:].ap[0], [0, K], [D, H], [1, D]])
                    nc.vector.tensor_tensor(out=prod[:], in0=kf, in1=qb,
                                            op=mybir.AluOpType.mult)
                    logits = psm.tile([P, K, H], F32, tag="log")
                    nc.vector.tensor_reduce(out=logits[:], in_=prod[:],
                                            axis=mybir.AxisListType.X,
                                            op=mybir.AluOpType.add)
                    nc.vector.tensor_tensor(out=logits[:], in0=logits[:],
                                            in1=bias_ps[:],
                                            op=mybir.AluOpType.add)
                    b2b = bass.AP(tensor=b2_sb.tensor, offset=b2_sb[:].offset,
                                  ap=[b2_sb[:].ap[0], [0, K], [1, H]])
                    nc.vector.tensor_tensor(out=logits[:], in0=logits[:],
                                            in1=b2b, op=mybir.AluOpType.add)

                    # --- softmax over k (logits bounded; skip max-sub)
                    pex = psm.tile([P, K, H], F32, tag="pex")
                    nc.scalar.activation(out=pex[:], in_=logits[:],
                                         func=mybir.ActivationFunctionType.Exp)
                    ssum = psm.tile([P, H], F32, tag="ssum")
                    pex_hk = bass.AP(tensor=pex.tensor, offset=pex[:].offset,
                                     ap=[pex[:].ap[0], [1, H], [H, K]])
                    nc.vector.tensor_reduce(out=ssum[:], in_=pex_hk,
                                            axis=mybir.AxisListType.X,
                                            op=mybir.AluOpType.add)
                    rinv = psm.tile([P, H], F32, tag="rinv")
                    nc.vector.reciprocal(out=rinv[:], in_=ssum[:])
                    attn = pb.tile([P, K, H], BF16, tag="attn")
                    rib = bass.AP(tensor=rinv.tensor, offset=rinv[:].offset,
                                  ap=[rinv[:].ap[0], [0, K], [1, H]])
                    nc.vector.tensor_tensor(out=attn[:], in0=pex[:], in1=rib,
                                            op=mybir.AluOpType.mult)

                    # --- out = sum_k attn * v   (pairwise bf16 tree)
                    av = pkv.tile([P, K, C], BF16, tag="av")
                    for k in range(K):
                        vsl = bass.AP(tensor=kv.tensor,
                                      offset=kv[:].offset + k * ROW + C,
                                      ap=[kv[:].ap[0], [D, H], [1, D]])
                        asl = attn[:, k, :]
                        ab = bass.AP(tensor=asl.tensor, offset=asl.offset,
                                     ap=[asl.ap[0], [1, H], [0, D]])
                        nc.vector.tensor_tensor(
                            out=av[:, k, :].rearrange("p (h d) -> p h d",
                                                      h=H),
                            in0=vsl, in1=ab, op=mybir.AluOpType.mult)
                    stride = K // 2
                    while stride >= 1:
                        for k in range(stride):
                            nc.vector.tensor_tensor(
                                out=av[:, k, :], in0=av[:, k, :],
                                in1=av[:, k + stride, :],
                                op=mybir.AluOpType.add)
                        stride //= 2

                    # --- O projection
                    oT_ps = ps_tp.tile([P, 2, P], BF16, tag="tp")
                    for j in range(2):
                        nc.tensor.transpose(out=oT_ps[:, j, :],
                                            in_=av[:, 0, j * P:(j + 1) * P],
                                            identity=ident_bf[:])
                    oT = pb.tile([P, 2, P], BF16, tag="oT")
                    nc.scalar.copy(out=oT[:], in_=oT_ps[:])
                    ops = ps_mm.tile([P, C], F32, tag="mm")
                    for j in range(2):
                        nc.tensor.matmul(ops[:], lhsT=oT[:, j, :],
                                         rhs=wo_sb[:, j, :],
                                         start=(j == 0), stop=(j == 1))

                    # --- residual + layernorm
                    xr = pb.tile([P, C], F32, tag="xr")
                    nc.sync.dma_start(out=xr[:],
                                      in_=xres[t * P:(t + 1) * P, :])
                    y0 = pb.tile([P, C], F32, tag="y0")
                    nc.vector.tensor_tensor(out=y0[:], in0=ops[:],
                                            in1=bo_sb[:],
                                            op=mybir.AluOpType.add)
                    nc.vector.tensor_tensor(out=y0[:], in0=y0[:], in1=xr[:],
                                            op=mybir.AluOpType.add)
                    bst = psm.tile([P, 6], F32, tag="bst")
                    nc.vector.bn_stats(out=bst[:], in_=y0[:])
                    mv = psm.tile([P, 2], F32, tag="mv")
                    nc.vector.bn_aggr(out=mv[:], in_=bst[:])
                    std = psm.tile([P, 1], F32, tag="std")
                    nc.scalar.activation(out=std[:], in_=mv[:, 1:2],
                                         func=mybir.ActivationFunctionType.Sqrt,
                                         bias=eps_sb[:])
                    rstd = psm.tile([P, 1], F32, tag="rstd")
                    nc.vector.reciprocal(out=rstd[:], in_=std[:])
                    nc.vector.tensor_scalar(out=y0[:], in0=y0[:],
                                            scalar1=mv[:, 0:1],
                                            scalar2=rstd[:],
                                            op0=mybir.AluOpType.subtract,
                                            op1=mybir.AluOpType.mult)
                    yt = pb.tile([P, C], F32, tag="yt")
                    nc.vector.tensor_tensor(out=yt[:], in0=y0[:],
                                            in1=gam_sb[:],
                                            op=mybir.AluOpType.mult)
                    nc.vector.tensor_tensor(out=yt[:], in0=yt[:],
                                            in1=bet_sb[:],
                                            op=mybir.AluOpType.add)
                    nc.sync.dma_start(out=y[t * P:(t + 1) * P, :], in_=yt[:])
                    if dbg and t == 0:
                        nc.sync.dma_start(out=dbg_kv[:, :],
                                          in_=kv[:].rearrange(
                                              "p a b -> p (a b)"))
                        nc.sync.dma_start(out=dbg_q[:, :], in_=q_sb[:])
                        nc.sync.dma_start(out=dbg_log[:, :],
                                          in_=logits[:].rearrange(
                                              "p a b -> p (a b)"))
                        nc.sync.dma_start(out=dbg_attn[:, :],
                                          in_=attn[:].rearrange(
                                              "p a b -> p (a b)"))
                        nc.sync.dma_start(out=dbg_av[:, :], in_=av[:, 0, :])
                        nc.sync.dma_start(out=dbg_hid[:, :], in_=hid_sb[:])
                        nc.sync.dma_start(out=dbg_rel[:, :],
                                          in_=rel[:].rearrange(
                                              "p a b -> p (a b)"))

    nc.compile()
    return nc


# ------------------------------------------------------------------ driver

_NC_CACHE = {}
RUN_KWARGS = {}
LAST_RESULT = None


def _get_nc(cfg_key, cfg):
    if cfg_key not in _NC_CACHE:
        _NC_CACHE[cfg_key] = build_nc(cfg)
    return _NC_CACHE[cfg_key]


def kernel(**inputs):
    global LAST_RESULT
    from concourse.bass_utils import run_bass_kernel_spmd

    cfg = full_cfg()
    in_maps = host_prep(inputs, cfg)
    nc = _get_nc("full", cfg)
    res = run_bass_kernel_spmd(nc, in_maps, core_ids=list(range(NCORES)),
                               **RUN_KWARGS)
    LAST_RESULT = res
    y = np.concatenate([res.results[c]["y"] for c in range(NCORES)], 0)
    return np.ascontiguousarray(y[:N])


if __name__ == "__main__":
    pass



# revision 10
# speedup vs baseline: 21.7264x; 21.7264x over previous
"""Trainium2 Bass kernel for nn_LocalAttention (sparse point-cloud attention).

Self-contained: accepts FULL unsharded inputs, shards across 8 NeuronCores
internally, returns the FULL (50000, 256) float32 output.

Distribution: points (N) sharded across 8 cores in 6272-row slices. Neighbor
indices are uniform over N, so each core builds the full neighbor table
[K(x) | V(x) | pos] in its own HBM (phase A), then runs attention for its own
shard (phase B). Weights replicated.

Gather strategy (phase B): the two SWDGE gather primitives use different
descriptor rings, so running both concurrently nearly doubles random-HBM
gather throughput:
  - neighbor slots 0..7  -> indirect_dma_start, one 576-elem row per edge
  - neighbor slots 8..15 -> dma_gather of PAIR rows (1152 elems); int16 pair
    indices (idx>>1 < 25088) satisfy the primitive's index-dtype limit. Both
    halves of each pair participate in attention; the wrong half is killed
    with a -30000 logit mask (host-precomputed from idx parity).
Attention therefore runs over S = 8 + 16 = 24 slots.

Math folds (exact, host-side):
  - softmax(l + c) == softmax(l)  -> bk and b2 drop out of logits entirely.
  - sum_k attn = 1                -> bv contributes bv @ Wo, folded with bo
    into the residual input rows (xres_b = x + bv@Wo + bo).
  - logits scale 1/sqrt(D)        -> folded into Wq, bq.
  - b1 folded into the rel-MLP via a ones-row appended to transposed rel.
"""

import sys

import numpy as np

for _p in ("/opt/trn_rl_repo",):
    if _p not in sys.path:
        sys.path.insert(0, _p)

import ml_dtypes

import concourse.bass as bass
import concourse.tile as tile
from concourse import bacc, library_config, mybir
from concourse.bass import IndirectOffsetOnAxis
from concourse.masks import make_identity

BF16 = mybir.dt.bfloat16
F32 = mybir.dt.float32
I32 = mybir.dt.int32
I16 = mybir.dt.int16

N, C, H, D, K = 50000, 256, 8, 8 * 4, 16
assert C == H * D
SCALE = D**-0.5
EPS = 1e-5
NCORES = 8
P = 128
SROW = 576          # table row: 256 k | 256 v | 6 pos-bits | 58 pad (bf16)
PROW = 2 * SROW
NSI = 8             # slots gathered as singles (indirect ring)
NSP = K - NSI       # slots gathered as pairs (dma_gather ring)
NCH = NSP * P // 512  # 512-index dma_gather chunks per tile
S = NSI + 2 * NSP   # attention slots (pair halves masked)
MASKVAL = -30000.0

nbf16 = ml_dtypes.bfloat16


def full_cfg():
    SH = 6272  # points per core (49 * 128); last core only 6096 valid
    return dict(NPAD=NCORES * SH, SH=SH, T=SH // P, G=NCORES * SH // P,
                ncores=NCORES)


# ---------------------------------------------------------------- host prep


def host_prep(inputs, cfg):
    """Full np inputs -> per-core in_maps for the SPMD kernel."""
    NPAD, SH, T, G = cfg["NPAD"], cfg["SH"], cfg["T"], cfg["G"]
    ncores = cfg["ncores"]
    n = inputs["x"].shape[0]

    x = np.asarray(inputs["x"], np.float32)
    pos = np.asarray(inputs["pos"], np.float32)
    idx = np.asarray(inputs["idx"]).astype(np.int64)
    Wq = np.asarray(inputs["Wq"], np.float32)
    bq = np.asarray(inputs["bq"], np.float32)
    Wk = np.asarray(inputs["Wk"], np.float32)
    Wv = np.asarray(inputs["Wv"], np.float32)
    Wo = np.asarray(inputs["Wo"], np.float32)
    bo = np.asarray(inputs["bo"], np.float32)
    bv = np.asarray(inputs["bv"], np.float32)
    W1 = np.asarray(inputs["W1"], np.float32)
    b1 = np.asarray(inputs["b1"], np.float32)
    W2 = np.asarray(inputs["W2"], np.float32)
    gamma = np.asarray(inputs["gamma"], np.float32)
    beta = np.asarray(inputs["beta"], np.float32)

    xpad = np.zeros((NPAD, C), np.float32)
    xpad[:n] = x
    pospad = np.zeros((NPAD, 3), np.float32)
    pospad[:n] = pos
    idxpad = np.zeros((NPAD, K), np.int64)
    idxpad[:n] = idx

    # x^T tiles for PE lhsT: [g, p, j, m] = xpad[g*128 + m, j*128 + p]
    a = xpad.reshape(G, P, 2, P)  # [g, m, j, p]
    xtt = np.ascontiguousarray(a.transpose(0, 3, 2, 1)).astype(nbf16)

    # table init: zeros except host-filled pos bits at cols 512..517
    tbl0 = np.zeros((NPAD, SROW), np.uint16)
    tbl0[:, 512:518] = pospad.view(np.uint16).reshape(NPAD, 6)
    tbl0 = tbl0.view(nbf16)

    # weights: (128, 2, 256) layout -> w[c, j, o] = W[j*128 + c, o]
    def wlay(W):
        return np.ascontiguousarray(
            W.reshape(2, P, C).transpose(1, 0, 2)).astype(nbf16)

    wq_l = wlay(Wq * SCALE)
    wk_l = wlay(Wk)
    wv_l = wlay(Wv)
    wo_l = wlay(Wo)
    # rel-MLP as block-diagonal matmul over rows (i|1, s) -> cols (s, j)
    w1aug = np.concatenate([W1, b1[None, :]], 0).astype(np.float32)  # (4, 64)
    w1big = np.zeros((4 * S, S * 64), np.float32)
    for s in range(S):
        for i in range(4):
            w1big[i * S + s, s * 64:(s + 1) * 64] = w1aug[i]
    w1big = w1big.astype(nbf16)
    # hidT block rows (kl, j) kl in {0,1} -> cols (kl, h)
    w2big = np.zeros((P, 2 * H), np.float32)
    for kl in range(2):
        w2big[kl * 64:(kl + 1) * 64, kl * H:(kl + 1) * H] = W2
    w2big = w2big.astype(nbf16)
    bq_s = (bq * SCALE).astype(np.float32)
    boeff = (bv @ Wo + bo).astype(np.float32)

    shared = dict(xtt=xtt, tbl0=tbl0, wq=wq_l, wk=wk_l, wv=wv_l,
                  wo=wo_l, w1b=w1big, w2=w2big, bq=bq_s,
                  gamma=gamma, beta=beta)

    in_maps = []
    for c in range(ncores):
        lo = c * SH
        sl = slice(lo, lo + SH)
        idxc = idxpad[sl].reshape(T, P, K)  # [t, p, k]
        # singles: [p, t*NSI + k] for slots 0..NSI-1
        idxr = np.ascontiguousarray(
            idxc[:, :, :NSI].transpose(1, 0, 2)).reshape(P, T * NSI)
        idxr = idxr.astype(np.int32)
        # pairs: slots NSI..K-1, int16 pair indices, wrapped+replicated
        pidx = (idxc[:, :, NSI:] >> 1).astype(np.int16)  # [t, p, m]
        idx16 = np.zeros((P, T * NCH * 32), np.int16)
        ii = np.arange(512)
        for t in range(T):
            for j in range(NCH):
                blk = np.zeros((16, 32), np.int16)
                blk[ii % 16, ii // 16] = pidx[t, ii % 128, 4 * j + ii // 128]
                idx16[:, (t * NCH + j) * 32:(t * NCH + j + 1) * 32] = \
                    np.tile(blk, (8, 1))
        # mask over attention slots: singles 0, pair halves by idx parity
        mask = np.zeros((T, P, S), np.float32)
        par = (idxc[:, :, NSI:] & 1).astype(np.int64)  # [t, p, m]
        for m in range(NSP):
            mask[:, :, NSI + 2 * m] = np.where(par[:, :, m] == 0, 0.0,
                                               MASKVAL)
            mask[:, :, NSI + 2 * m + 1] = np.where(par[:, :, m] == 1, 0.0,
                                                   MASKVAL)
        maskr = np.ascontiguousarray(
            mask.transpose(1, 0, 2)).reshape(P, T * S)
        posr = np.ascontiguousarray(
            pospad[sl].reshape(T, P, 3).transpose(1, 0, 2)).reshape(P, T * 3)
        xres_b = np.ascontiguousarray(xpad[sl]) + boeff[None, :]
        m = dict(shared)
        m.update(xres=xres_b.astype(np.float32),
                 xqt=np.ascontiguousarray(xtt[c * T:(c + 1) * T]),
                 idxr=idxr, idx16=idx16, maskr=maskr, posr=posr)
        in_maps.append(m)
    return in_maps


# ------------------------------------------------------------- bass program


def build_nc(cfg):
    NPAD, SH, T, G = cfg["NPAD"], cfg["SH"], cfg["T"], cfg["G"]
    NPAIR = NPAD // 2

    nc = bacc.Bacc(trn_type="TRN2")

    # I/O
    xtt = nc.dram_tensor("xtt", [G, P, 2, P], BF16, kind="ExternalInput")
    tbl = nc.dram_tensor("tbl0", [NPAD, SROW], BF16, kind="ExternalInput")
    xqt = nc.dram_tensor("xqt", [T, P, 2, P], BF16, kind="ExternalInput")
    xres = nc.dram_tensor("xres", [SH, C], F32, kind="ExternalInput")
    idxr = nc.dram_tensor("idxr", [P, T * NSI], I32, kind="ExternalInput")
    idx16 = nc.dram_tensor("idx16", [P, T * NCH * 32], I16,
                           kind="ExternalInput")
    maskr = nc.dram_tensor("maskr", [P, T * S], F32, kind="ExternalInput")
    posr = nc.dram_tensor("posr", [P, T * 3], F32, kind="ExternalInput")
    wq = nc.dram_tensor("wq", [P, 2, C], BF16, kind="ExternalInput")
    wk = nc.dram_tensor("wk", [P, 2, C], BF16, kind="ExternalInput")
    wv = nc.dram_tensor("wv", [P, 2, C], BF16, kind="ExternalInput")
    wo = nc.dram_tensor("wo", [P, 2, C], BF16, kind="ExternalInput")
    w1b = nc.dram_tensor("w1b", [4 * S, S * 64], BF16, kind="ExternalInput")
    w2 = nc.dram_tensor("w2", [P, 2 * H], BF16, kind="ExternalInput")
    bq = nc.dram_tensor("bq", [C], F32, kind="ExternalInput")
    gamma = nc.dram_tensor("gamma", [C], F32, kind="ExternalInput")
    beta = nc.dram_tensor("beta", [C], F32, kind="ExternalInput")
    y = nc.dram_tensor("y", [SH, C], F32, kind="ExternalOutput")

    dbg = cfg.get("dbg")
    if dbg:
        dbg_sg = nc.dram_tensor("dbg_sg", [P, NSI * SROW], BF16,
                                kind="ExternalOutput")
        dbg_pg = nc.dram_tensor("dbg_pg", [P, NSP * PROW], BF16,
                                kind="ExternalOutput")
        dbg_q = nc.dram_tensor("dbg_q", [P, C], BF16, kind="ExternalOutput")
        dbg_rel = nc.dram_tensor("dbg_rel", [P, 4 * S], F32,
                                 kind="ExternalOutput")
        dbg_hid = nc.dram_tensor("dbg_hid", [P, S * 64], BF16,
                                 kind="ExternalOutput")
        dbg_log = nc.dram_tensor("dbg_log", [P, S * H], F32,
                                 kind="ExternalOutput")
        dbg_attn = nc.dram_tensor("dbg_attn", [P, S * H], BF16,
                                  kind="ExternalOutput")
        dbg_av = nc.dram_tensor("dbg_av", [P, C], BF16,
                                kind="ExternalOutput")

    tblp = tbl[:, :].rearrange("(a b) c -> a (b c)", b=2)  # [NPAIR, PROW]

    def bcast_vec(v, cols):
        return bass.AP(tensor=v.ap().tensor, offset=0, ap=[[0, P], [1, cols]])

    with tile.TileContext(nc) as tc:
        import contextlib

        with contextlib.ExitStack() as ctx:
            consts = ctx.enter_context(tc.tile_pool(name="consts", bufs=1))

            nc.gpsimd.load_library(library_config.mlp)

            wq_sb = consts.tile([P, 2, C], BF16)
            wk_sb = consts.tile([P, 2, C], BF16)
            wv_sb = consts.tile([P, 2, C], BF16)
            wo_sb = consts.tile([P, 2, C], BF16)
            for t_sb, t_dr in ((wq_sb, wq), (wk_sb, wk), (wv_sb, wv),
                               (wo_sb, wo)):
                nc.sync.dma_start(out=t_sb[:], in_=t_dr[:, :, :])
            w1_sb = consts.tile([4 * S, S * 64], BF16)
            nc.sync.dma_start(out=w1_sb[:], in_=w1b[:, :])
            w2_sb = consts.tile([P, 2 * H], BF16)
            nc.sync.dma_start(out=w2_sb[:], in_=w2[:, :])
            idxr_sb = consts.tile([P, T * NSI], I32)
            nc.sync.dma_start(out=idxr_sb[:], in_=idxr[:, :])
            idx16_sb = consts.tile([P, T * NCH * 32], I16)
            nc.sync.dma_start(out=idx16_sb[:], in_=idx16[:, :])
            mask_sb = consts.tile([P, T * S], F32)
            nc.sync.dma_start(out=mask_sb[:], in_=maskr[:, :])
            posr_sb = consts.tile([P, T * 3], F32)
            nc.sync.dma_start(out=posr_sb[:], in_=posr[:, :])
            bq_sb = consts.tile([P, C], F32)
            nc.sync.dma_start(out=bq_sb[:], in_=bcast_vec(bq, C))
            gam_sb = consts.tile([P, C], F32)
            nc.sync.dma_start(out=gam_sb[:], in_=bcast_vec(gamma, C))
            bet_sb = consts.tile([P, C], F32)
            nc.sync.dma_start(out=bet_sb[:], in_=bcast_vec(beta, C))
            eps_sb = consts.tile([P, 1], F32)
            nc.vector.memset(eps_sb[:], EPS)
            ident = consts.tile([P, P], F32)
            make_identity(nc, ident[:])
            ident_bf = consts.tile([P, P], BF16)
            nc.vector.tensor_copy(out=ident_bf[:], in_=ident[:])

            # ---------------- phase A: neighbor table [k | v] cols 0..511
            with tc.tile_pool(name="pa", bufs=4) as pa, \
                    tc.tile_pool(name="paps", bufs=2, space="PSUM") as paps:
                for g in range(G):
                    xt = pa.tile([P, 2, P], BF16, tag="xt")
                    nc.sync.dma_start(out=xt[:], in_=xtt[g, :, :, :])
                    row = pa.tile([P, 2 * C], BF16, tag="row")
                    kps = paps.tile([P, C], F32, tag="kps")
                    vps = paps.tile([P, C], F32, tag="vps")
                    for j in range(2):
                        nc.tensor.matmul(kps[:], lhsT=xt[:, j, :],
                                         rhs=wk_sb[:, j, :],
                                         start=(j == 0), stop=(j == 1))
                    for j in range(2):
                        nc.tensor.matmul(vps[:], lhsT=xt[:, j, :],
                                         rhs=wv_sb[:, j, :],
                                         start=(j == 0), stop=(j == 1))
                    nc.scalar.copy(out=row[:, 0:C], in_=kps[:])
                    nc.scalar.copy(out=row[:, C:2 * C], in_=vps[:])
                    nc.sync.dma_start(
                        out=tbl[g * P:(g + 1) * P, 0:2 * C], in_=row[:])

            # ---------------- phase B: local attention on own shard
            with contextlib.ExitStack() as bctx:
                pb = bctx.enter_context(tc.tile_pool(name="pb", bufs=2))
                pkv = bctx.enter_context(tc.tile_pool(name="pkv", bufs=2))
                ps_mm = bctx.enter_context(
                    tc.tile_pool(name="ps_mm", bufs=2, space="PSUM"))
                ps_tp = bctx.enter_context(
                    tc.tile_pool(name="ps_tp", bufs=2, space="PSUM"))
                ps_h = bctx.enter_context(
                    tc.tile_pool(name="ps_h", bufs=2, space="PSUM"))
                ps_b = bctx.enter_context(
                    tc.tile_pool(name="ps_b", bufs=1, space="PSUM"))

                for t in range(T):
                    # --- gathers: singles (indirect ring) + pairs (tq ring)
                    sg = pkv.tile([P, NSI, SROW], BF16, tag="sg")
                    pg = pkv.tile([P, NSP, PROW], BF16, tag="pg")
                    for j in range(NCH):
                        nc.gpsimd.dma_gather(
                            out_ap=pg[:, 4 * j:4 * (j + 1), :],
                            in_ap=tblp,
                            idxs_ap=idx16_sb[:, (t * NCH + j) * 32:
                                             (t * NCH + j + 1) * 32],
                            num_idxs=512,
                            num_idxs_reg=512,
                            elem_size=PROW,
                        )
                    for k in range(NSI):
                        nc.gpsimd.indirect_dma_start(
                            out=sg[:, k, :], out_offset=None,
                            in_=tbl[:, :],
                            in_offset=IndirectOffsetOnAxis(
                                ap=idxr_sb[:, t * NSI + k:t * NSI + k + 1],
                                axis=0))

                    # --- Q projection (pre-scaled)
                    xq = pb.tile([P, 2, P], BF16, tag="xq")
                    nc.sync.dma_start(out=xq[:], in_=xqt[t, :, :, :])
                    qps = ps_mm.tile([P, C], F32, tag="mm")
                    for j in range(2):
                        nc.tensor.matmul(qps[:], lhsT=xq[:, j, :],
                                         rhs=wq_sb[:, j, :],
                                         start=(j == 0), stop=(j == 1))
                    q_sb = pb.tile([P, C], BF16, tag="q")
                    nc.vector.tensor_tensor(out=q_sb[:], in0=qps[:],
                                            in1=bq_sb[:],
                                            op=mybir.AluOpType.add)

                    # --- rel = pos_nb - pos (f32 views of pos bits)
                    rel = pb.tile([P, 4, S], F32, tag="rel")
                    nc.vector.memset(rel[:, 3, :], 1.0)
                    psl = posr_sb[:, t * 3:(t + 1) * 3]
                    sgf = sg[:].bitcast(F32)   # [P, NSI, 288]
                    pnb_s = bass.AP(tensor=sgf.tensor, offset=sgf.offset + 256,
                                    ap=[sgf.ap[0], [1, 3], [288, NSI]])
                    pbr_s = bass.AP(tensor=psl.tensor, offset=psl.offset,
                                    ap=[psl.ap[0], [1, 3], [0, NSI]])
                    nc.vector.tensor_tensor(out=rel[:, 0:3, 0:NSI],
                                            in0=pnb_s, in1=pbr_s,
                                            op=mybir.AluOpType.subtract)
                    pgf = pg[:].bitcast(F32)   # [P, NSP, 1152=2*576 f32]
                    pnb_p = bass.AP(tensor=pgf.tensor, offset=pgf.offset + 256,
                                    ap=[pgf.ap[0], [1, 3], [288, 2 * NSP]])
                    pbr_p = bass.AP(tensor=psl.tensor, offset=psl.offset,
                                    ap=[psl.ap[0], [1, 3], [0, 2 * NSP]])
                    nc.vector.tensor_tensor(out=rel[:, 0:3, NSI:S],
                                            in0=pnb_p, in1=pbr_p,
                                            op=mybir.AluOpType.subtract)

                    # --- rel MLP: hid = gelu([rel|1] @ [W1;b1])
                    relT_ps = ps_tp.tile([4 * S, P], F32, tag="tp")
                    rel2d = rel[:].rearrange("p a s -> p (a s)")
                    nc.tensor.transpose(out=relT_ps[:], in_=rel2d,
                                        identity=ident[:])
                    relT = pb.tile([4 * S, P], BF16, tag="relT")
                    nc.scalar.copy(out=relT[:], in_=relT_ps[:])
                    hid_sb = pb.tile([P, S * 64], BF16, tag="hid_sb")
                    for j in range(3):
                        hid_ps = ps_h.tile([P, 512], F32, tag="hid")
                        nc.tensor.matmul(hid_ps[:],
                                         lhsT=relT[:],
                                         rhs=w1_sb[:, j * 512:(j + 1) * 512],
                                         start=True, stop=True)
                        nc.scalar.activation(
                            out=hid_sb[:, j * 512:(j + 1) * 512],
                            in_=hid_ps[:],
                            func=mybir.ActivationFunctionType.Gelu)
                    nblk = S // 2
                    hidT = pb.tile([P, nblk, P], BF16, tag="hidT")
                    for hh in range(2):
                        hidT_ps = ps_tp.tile([P, nblk // 2, P], BF16,
                                             tag="tp")
                        for b in range(nblk // 2):
                            bb = hh * (nblk // 2) + b
                            nc.tensor.transpose(
                                out=hidT_ps[:, b, :],
                                in_=hid_sb[:, bb * P:(bb + 1) * P],
                                identity=ident_bf[:])
                        nc.scalar.copy(
                            out=hidT[:, hh * (nblk // 2):(hh + 1) * (nblk // 2), :],
                            in_=hidT_ps[:])
                    bias_ps = ps_b.tile([P, S, H], F32, tag="bias")
                    for b in range(nblk):
                        nc.tensor.matmul(
                            bias_ps[:, 2 * b:2 * b + 2, :].rearrange(
                                "p a h -> p (a h)"),
                            lhsT=hidT[:, b, :], rhs=w2_sb[:],
                            start=True, stop=True)

                    # --- logits = q . k_nb  (+ mlp bias + mask), (s, h)
                    prod = pkv.tile([P, S, H, D], BF16, tag="prod")
                    kf_s = bass.AP(tensor=sg.tensor, offset=sg[:].offset,
                                   ap=[sg[:].ap[0], [SROW, NSI], [D, H],
                                       [1, D]])
                    qb_s = bass.AP(tensor=q_sb.tensor, offset=q_sb[:].offset,
                                   ap=[q_sb[:].ap[0], [0, NSI], [D, H],
                                       [1, D]])
                    nc.vector.tensor_tensor(out=prod[:, 0:NSI], in0=kf_s,
                                            in1=qb_s,
                                            op=mybir.AluOpType.mult)
                    kf_p = bass.AP(tensor=pg.tensor, offset=pg[:].offset,
                                   ap=[pg[:].ap[0], [SROW, 2 * NSP],
                                       [D, H], [1, D]])
                    qb_p = bass.AP(tensor=q_sb.tensor, offset=q_sb[:].offset,
                                   ap=[q_sb[:].ap[0], [0, 2 * NSP],
                                       [D, H], [1, D]])
                    nc.vector.tensor_tensor(out=prod[:, NSI:S],
                                            in0=kf_p, in1=qb_p,
                                            op=mybir.AluOpType.mult)
                    logits = pb.tile([P, S, H], F32, tag="log")
                    nc.vector.tensor_reduce(out=logits[:], in_=prod[:],
                                            axis=mybir.AxisListType.X,
                                            op=mybir.AluOpType.add)
                    nc.vector.tensor_tensor(out=logits[:], in0=logits[:],
                                            in1=bias_ps[:],
                                            op=mybir.AluOpType.add)
                    msl = mask_sb[:, t * S:(t + 1) * S]
                    mb = bass.AP(tensor=msl.tensor, offset=msl.offset,
                                 ap=[msl.ap[0], [1, S], [0, H]])
                    nc.vector.tensor_tensor(out=logits[:], in0=logits[:],
                                            in1=mb, op=mybir.AluOpType.add)

                    # --- softmax over s (logits bounded; skip max-sub)
                    pex = pb.tile([P, S, H], F32, tag="pex")
                    nc.scalar.activation(out=pex[:], in_=logits[:],
                                         func=mybir.ActivationFunctionType.Exp)
                    ssum = pb.tile([P, H], F32, tag="ssum")
                    pex_hs = bass.AP(tensor=pex.tensor, offset=pex[:].offset,
                                     ap=[pex[:].ap[0], [1, H], [H, S]])
                    nc.vector.tensor_reduce(out=ssum[:], in_=pex_hs,
                                            axis=mybir.AxisListType.X,
                                            op=mybir.AluOpType.add)
                    rinv = pb.tile([P, H], F32, tag="rinv")
                    nc.vector.reciprocal(out=rinv[:], in_=ssum[:])
                    attn = pb.tile([P, S, H], BF16, tag="attn")
                    rib = bass.AP(tensor=rinv.tensor, offset=rinv[:].offset,
                                  ap=[rinv[:].ap[0], [0, S], [1, H]])
                    nc.vector.tensor_tensor(out=attn[:], in0=pex[:], in1=rib,
                                            op=mybir.AluOpType.mult)

                    # --- av[s] = attn[s] * v[s]; then tree-sum over s
                    av = pkv.tile([P, S, C], BF16, tag="av")
                    vf_s = bass.AP(tensor=sg.tensor,
                                   offset=sg[:].offset + C,
                                   ap=[sg[:].ap[0], [SROW, NSI], [D, H],
                                       [1, D]])
                    at_s = bass.AP(tensor=attn.tensor, offset=attn[:].offset,
                                   ap=[attn[:].ap[0], [H, NSI], [1, H],
                                       [0, D]])
                    nc.vector.tensor_tensor(
                        out=av[:, 0:NSI].rearrange("p s (h d) -> p s h d",
                                                   h=H),
                        in0=vf_s, in1=at_s, op=mybir.AluOpType.mult)
                    vf_p = bass.AP(tensor=pg.tensor,
                                   offset=pg[:].offset + C,
                                   ap=[pg[:].ap[0], [SROW, 2 * NSP],
                                       [D, H], [1, D]])
                    at_p = bass.AP(tensor=attn.tensor,
                                   offset=attn[:].offset + NSI * H,
                                   ap=[attn[:].ap[0], [H, 2 * NSP],
                                       [1, H], [0, D]])
                    nc.vector.tensor_tensor(
                        out=av[:, NSI:S].rearrange("p s (h d) -> p s h d",
                                                   h=H),
                        in0=vf_p, in1=at_p, op=mybir.AluOpType.mult)
                    half = S // 2
                    while half >= 3:
                        nc.vector.tensor_tensor(
                            out=av[:, 0:half], in0=av[:, 0:half],
                            in1=av[:, half:2 * half],
                            op=mybir.AluOpType.add)
                        half //= 2
                    nc.vector.tensor_tensor(out=av[:, 0:1], in0=av[:, 0:1],
                                            in1=av[:, 1:2],
                                            op=mybir.AluOpType.add)
                    nc.vector.tensor_tensor(out=av[:, 0:1], in0=av[:, 0:1],
                                            in1=av[:, 2:3],
                                            op=mybir.AluOpType.add)

                    # --- O projection
                    oT_ps = ps_tp.tile([P, 2, P], BF16, tag="tp")
                    for j in range(2):
                        nc.tensor.transpose(out=oT_ps[:, j, :],
                                            in_=av[:, 0, j * P:(j + 1) * P],
                                            identity=ident_bf[:])
                    oT = pb.tile([P, 2, P], BF16, tag="oT")
                    nc.scalar.copy(out=oT[:], in_=oT_ps[:])
                    ops = ps_mm.tile([P, C], F32, tag="mm")
                    for j in range(2):
                        nc.tensor.matmul(ops[:], lhsT=oT[:, j, :],
                                         rhs=wo_sb[:, j, :],
                                         start=(j == 0), stop=(j == 1))

                    # --- residual (+ folded bv@Wo + bo) + layernorm
                    xr = pb.tile([P, C], F32, tag="xr")
                    nc.sync.dma_start(out=xr[:],
                                      in_=xres[t * P:(t + 1) * P, :])
                    y0 = pb.tile([P, C], F32, tag="y0")
                    nc.vector.tensor_tensor(out=y0[:], in0=ops[:], in1=xr[:],
                                            op=mybir.AluOpType.add)
                    bst = pb.tile([P, 6], F32, tag="bst")
                    nc.vector.bn_stats(out=bst[:], in_=y0[:])
                    mv = pb.tile([P, 2], F32, tag="mv")
                    nc.vector.bn_aggr(out=mv[:], in_=bst[:])
                    std = pb.tile([P, 1], F32, tag="std")
                    nc.scalar.activation(out=std[:], in_=mv[:, 1:2],
                                         func=mybir.ActivationFunctionType.Sqrt,
                                         bias=eps_sb[:])
                    rstd = pb.tile([P, 1], F32, tag="rstd")
                    nc.vector.reciprocal(out=rstd[:], in_=std[:])
                    nc.vector.tensor_scalar(out=y0[:], in0=y0[:],
                                            scalar1=mv[:, 0:1],
                                            scalar2=rstd[:],
                                            op0=mybir.AluOpType.subtract,
                                            op1=mybir.AluOpType.mult)
                    yt = pb.tile([P, C], F32, tag="yt")
                    nc.vector.tensor_tensor(out=yt[:], in0=y0[:],
                                            in1=gam_sb[:],
                                            op=mybir.AluOpType.mult)
                    nc.vector.tensor_tensor(out=yt[:], in0=yt[:],
                                            in1=bet_sb[:],
                                            op=mybir.AluOpType.add)
                    nc.sync.dma_start(out=y[t * P:(t + 1) * P, :], in_=yt[:])
                    if dbg and t == 0:
                        nc.sync.dma_start(out=dbg_sg[:, :], in_=sg[:].rearrange(
                            "p a b -> p (a b)"))
                        nc.sync.dma_start(out=dbg_pg[:, :], in_=pg[:].rearrange(
                            "p a b -> p (a b)"))
                        nc.sync.dma_start(out=dbg_q[:, :], in_=q_sb[:])
                        nc.sync.dma_start(out=dbg_rel[:, :], in_=rel[:].rearrange(
                            "p a b -> p (a b)"))
                        nc.sync.dma_start(out=dbg_hid[:, :], in_=hid_sb[:])
                        nc.sync.dma_start(out=dbg_log[:, :], in_=logits[:].rearrange(
                            "p a b -> p (a b)"))
                        nc.sync.dma_start(out=dbg_attn[:, :], in_=attn[:].rearrange(
                            "p a b -> p (a b)"))
                        nc.sync.dma_start(out=dbg_av[:, :], in_=av[:, 0, :])

    nc.compile()
    return nc


# ------------------------------------------------------------------ driver

_NC_CACHE = {}
RUN_KWARGS = {}
LAST_RESULT = None


def _get_nc(cfg_key, cfg):
    if cfg_key not in _NC_CACHE:
        _NC_CACHE[cfg_key] = build_nc(cfg)
    return _NC_CACHE[cfg_key]


def kernel(**inputs):
    global LAST_RESULT
    from concourse.bass_utils import run_bass_kernel_spmd

    cfg = full_cfg()
    in_maps = host_prep(inputs, cfg)
    nc = _get_nc("full", cfg)
    res = run_bass_kernel_spmd(nc, in_maps, core_ids=list(range(NCORES)),
                               **RUN_KWARGS)
    LAST_RESULT = res
    y = np.concatenate([res.results[c]["y"] for c in range(NCORES)], 0)
    return np.ascontiguousarray(y[:N])


if __name__ == "__main__":
    pass


# revision 45
# speedup vs baseline: 33.3825x; 1.5365x over previous
"""Trainium2 Bass kernel for nn_LocalAttention (sparse point-cloud attention).

Self-contained: accepts FULL unsharded inputs, shards across 8 NeuronCores
internally, returns the FULL (50000, 256) float32 output.

Distribution: points (N) sharded across 8 cores in 6272-row slices. Neighbor
indices are uniform over N, so each core builds the full neighbor table
[K(x) | V(x) | pos] in its own HBM (phase A), then runs attention for its own
shard (phase B). Weights replicated.

Gather strategy (phase B): the two SWDGE gather primitives use different
descriptor rings, so running both concurrently nearly doubles random-HBM
gather throughput:
  - neighbor slots 0..7  -> indirect_dma_start, one 576-elem row per edge
  - neighbor slots 8..15 -> dma_gather of PAIR rows (1152 elems); int16 pair
    indices (idx>>1 < 25088) satisfy the primitive's index-dtype limit. Both
    halves of each pair participate in attention; the wrong half is killed
    with a -30000 logit mask (host-precomputed from idx parity).
Attention therefore runs over S = 8 + 16 = 24 slots.

Math folds (exact, host-side):
  - softmax(l + c) == softmax(l)  -> bk and b2 drop out of logits entirely.
  - sum_k attn = 1                -> bv contributes bv @ Wo, folded with bo
    into the residual input rows (xres_b = x + bv@Wo + bo).
  - logits scale 1/sqrt(D)        -> folded into Wq, bq.
  - b1 folded into the rel-MLP via a ones-row appended to transposed rel.
"""

import sys

import numpy as np

for _p in ("/opt/trn_rl_repo",):
    if _p not in sys.path:
        sys.path.insert(0, _p)

import ml_dtypes

import concourse.bass as bass
import concourse.tile as tile
from concourse import bacc, library_config, mybir
from concourse.bass import IndirectOffsetOnAxis
from concourse.masks import make_identity

BF16 = mybir.dt.bfloat16
F32 = mybir.dt.float32
I32 = mybir.dt.int32
I16 = mybir.dt.int16

N, C, H, D, K = 50000, 256, 8, 8 * 4, 16
assert C == H * D
SCALE = D**-0.5
EPS = 1e-5
NCORES = 8
P = 128
SROW = 512          # table row: 256 k | 256 v  (bf16)
PROW = 2 * SROW
NSI = 8             # slots gathered as singles (indirect ring)
NSP = K - NSI       # slots gathered as pairs (dma_gather ring)
NCH = NSP * P // 512  # 512-index dma_gather chunks per tile
S = NSI + 2 * NSP   # attention slots (pair halves masked)
MASKVAL = -30000.0

nbf16 = ml_dtypes.bfloat16


def full_cfg():
    SH = 6272  # points per core (49 * 128); last core only 6096 valid
    return dict(NPAD=NCORES * SH, SH=SH, T=SH // P, G=NCORES * SH // P,
                ncores=NCORES)


# ---------------------------------------------------------------- host prep


def host_prep(inputs, cfg):
    """Full np inputs -> per-core in_maps for the SPMD kernel."""
    NPAD, SH, T, G = cfg["NPAD"], cfg["SH"], cfg["T"], cfg["G"]
    ncores = cfg["ncores"]
    n = inputs["x"].shape[0]

    x = np.asarray(inputs["x"], np.float32)
    pos = np.asarray(inputs["pos"], np.float32)
    idx = np.asarray(inputs["idx"]).astype(np.int64)
    Wq = np.asarray(inputs["Wq"], np.float32)
    bq = np.asarray(inputs["bq"], np.float32)
    Wk = np.asarray(inputs["Wk"], np.float32)
    Wv = np.asarray(inputs["Wv"], np.float32)
    Wo = np.asarray(inputs["Wo"], np.float32)
    bo = np.asarray(inputs["bo"], np.float32)
    bv = np.asarray(inputs["bv"], np.float32)
    W1 = np.asarray(inputs["W1"], np.float32)
    b1 = np.asarray(inputs["b1"], np.float32)
    W2 = np.asarray(inputs["W2"], np.float32)
    gamma = np.asarray(inputs["gamma"], np.float32)
    beta = np.asarray(inputs["beta"], np.float32)

    xpad = np.zeros((NPAD, C), np.float32)
    xpad[:n] = x
    pospad = np.zeros((NPAD, 3), np.float32)
    pospad[:n] = pos
    idxpad = np.zeros((NPAD, K), np.int64)
    idxpad[:n] = idx

    # x^T tiles for PE lhsT: [g, p, j, m] = xpad[g*128 + m, j*128 + p]
    a = xpad.reshape(G, P, 2, P)  # [g, m, j, p]
    xtt = np.ascontiguousarray(a.transpose(0, 3, 2, 1)).astype(nbf16)
    # phase-A quad-tile layout: [g4, p, u, j, m] = xtt[4*g4+u, p, j, m]
    xtt2 = np.ascontiguousarray(
        xtt.reshape(G // 4, 4, P, 2, P).transpose(0, 2, 1, 3, 4))



    # weights: (128, 2, 256) layout -> w[c, j, o] = W[j*128 + c, o]
    def wlay(W):
        return np.ascontiguousarray(
            W.reshape(2, P, C).transpose(1, 0, 2)).astype(nbf16)

    wq_l = wlay(Wq * SCALE)
    wk_l = wlay(Wk)
    wv_l = wlay(Wv)
    wo_l = wlay(Wo)
    wkv_l = np.concatenate([wk_l, wv_l], axis=-1)  # [P, 2, 2C]
    # rel-MLP as block-diagonal matmul over rows (i|1, s) -> cols (s, j)
    w1aug = np.concatenate([W1, b1[None, :]], 0).astype(np.float32)  # (4, 64)
    w1big = np.zeros((4 * S, S * 64), np.float32)
    for s in range(S):
        for i in range(4):
            w1big[i * S + s, s * 64:(s + 1) * 64] = w1aug[i]
    w1big = w1big.astype(nbf16)
    # hidT block rows (kl, j) kl in {0,1} -> cols (kl, h)
    w2big = np.zeros((P, 2 * H), np.float32)
    for kl in range(2):
        w2big[kl * 64:(kl + 1) * 64, kl * H:(kl + 1) * H] = W2
    w2big = w2big.astype(nbf16)
    bq_s = (bq * SCALE).astype(np.float32)
    boeff = (bv @ Wo + bo).astype(np.float32)

    shared = dict(xtt2=xtt2, wq=wq_l, wkv=wkv_l,
                  wo=wo_l, w1b=w1big, w2=w2big, bq=bq_s,
                  gamma=gamma, beta=beta)

    in_maps = []
    for c in range(ncores):
        lo = c * SH
        sl = slice(lo, lo + SH)
        idxc = idxpad[sl].reshape(T, P, K)  # [t, p, k]
        # singles: [p, t*NSI + k] for slots 0..NSI-1
        idxr = np.ascontiguousarray(
            idxc[:, :, :NSI].transpose(1, 0, 2)).reshape(P, T * NSI)
        idxr = idxr.astype(np.int32)
        # pairs: slots NSI..K-1, int16 pair indices, wrapped+replicated
        pidx = (idxc[:, :, NSI:] >> 1).astype(np.int16)  # [t, p, m]
        idx16 = np.zeros((P, T * NCH * 32), np.int16)
        ii = np.arange(512)
        for t in range(T):
            for j in range(NCH):
                blk = np.zeros((16, 32), np.int16)
                blk[ii % 16, ii // 16] = pidx[t, ii % 128, 4 * j + ii // 128]
                idx16[:, (t * NCH + j) * 32:(t * NCH + j + 1) * 32] = \
                    np.tile(blk, (8, 1))
        # mask over attention slots: singles 0, pair halves by idx parity
        mask = np.zeros((T, P, S), np.float32)
        par = (idxc[:, :, NSI:] & 1).astype(np.int64)  # [t, p, m]
        for m in range(NSP):
            mask[:, :, NSI + 2 * m] = np.where(par[:, :, m] == 0, 0.0,
                                               MASKVAL)
            mask[:, :, NSI + 2 * m + 1] = np.where(par[:, :, m] == 1, 0.0,
                                                   MASKVAL)
        maskr = np.ascontiguousarray(
            mask.transpose(1, 0, 2)).reshape(P, T * S)
        # rel-MLP input, host-side (pure function of pos+idx): for each
        # attention slot s the neighbor j(s); rows (i|1) pre-TRANSPOSED for
        # the PE lhsT: relmT[i*S+s, t*128+p] = (pos[j]-pos[p])[i] (i<3) | 1.
        jidx = np.zeros((T, P, S), np.int64)
        jidx[:, :, :NSI] = idxc[:, :, :NSI]
        for m_ in range(NSP):
            base = 2 * (idxc[:, :, NSI + m_] >> 1)
            jidx[:, :, NSI + 2 * m_] = base
            jidx[:, :, NSI + 2 * m_ + 1] = base + 1
        relf = (pospad[jidx] -
                pospad[sl].reshape(T, P, 1, 3))        # [t, p, s, 3]
        relmT = np.ones((4, S, T, P), np.float32)
        relmT[:3] = relf.transpose(3, 2, 0, 1)
        relmT = relmT.reshape(4 * S, T * P).astype(nbf16)
        xres_b = np.ascontiguousarray(xpad[sl]) + boeff[None, :]
        m = dict(shared)
        m.update(xres=xres_b.astype(np.float32),
                 xqt=np.ascontiguousarray(xtt[c * T:(c + 1) * T]),
                 idxr=idxr, idx16=idx16, maskr=maskr,
                 relmT=np.ascontiguousarray(relmT))
        in_maps.append(m)
    return in_maps


# ------------------------------------------------------------- bass program


def build_nc(cfg):
    NPAD, SH, T, G = cfg["NPAD"], cfg["SH"], cfg["T"], cfg["G"]
    NPAIR = NPAD // 2
    # ablation switches (timing experiments): full kernel when empty
    ablate = cfg.get("ablate", "")
    ab_nophaseB = "nophaseB" in ablate
    ab_nocompute = "nocompute" in ablate
    ab_nogather = "nogather" in ablate
    ab_Anoload = "Anoload" in ablate
    ab_Anostore = "Anostore" in ablate
    ab_Anomm = "Anomm" in ablate

    nc = bacc.Bacc(trn_type="TRN2")

    # I/O
    xtt2 = nc.dram_tensor("xtt2", [G // 4, P, 4, 2, P], BF16,
                          kind="ExternalInput")
    tbl = nc.dram_tensor("tbl", [NPAD, SROW], BF16)
    xqt = nc.dram_tensor("xqt", [T, P, 2, P], BF16, kind="ExternalInput")
    xres = nc.dram_tensor("xres", [SH, C], F32, kind="ExternalInput")
    idxr = nc.dram_tensor("idxr", [P, T * NSI], I32, kind="ExternalInput")
    idx16 = nc.dram_tensor("idx16", [P, T * NCH * 32], I16,
                           kind="ExternalInput")
    maskr = nc.dram_tensor("maskr", [P, T * S], F32, kind="ExternalInput")
    relmT = nc.dram_tensor("relmT", [4 * S, T * P], BF16,
                           kind="ExternalInput")
    wq = nc.dram_tensor("wq", [P, 2, C], BF16, kind="ExternalInput")
    wkv = nc.dram_tensor("wkv", [P, 2, 2 * C], BF16, kind="ExternalInput")
    wo = nc.dram_tensor("wo", [P, 2, C], BF16, kind="ExternalInput")
    w1b = nc.dram_tensor("w1b", [4 * S, S * 64], BF16, kind="ExternalInput")
    w2 = nc.dram_tensor("w2", [P, 2 * H], BF16, kind="ExternalInput")
    bq = nc.dram_tensor("bq", [C], F32, kind="ExternalInput")
    gamma = nc.dram_tensor("gamma", [C], F32, kind="ExternalInput")
    beta = nc.dram_tensor("beta", [C], F32, kind="ExternalInput")
    y = nc.dram_tensor("y", [SH, C], F32, kind="ExternalOutput")

    dbg = cfg.get("dbg")
    if dbg:
        dbg_sg = nc.dram_tensor("dbg_sg", [P, NSI * SROW], BF16,
                                kind="ExternalOutput")
        dbg_pg = nc.dram_tensor("dbg_pg", [P, NSP * PROW], BF16,
                                kind="ExternalOutput")
        dbg_q = nc.dram_tensor("dbg_q", [P, C], BF16, kind="ExternalOutput")
        dbg_hid = nc.dram_tensor("dbg_hid", [P, S * 64], BF16,
                                 kind="ExternalOutput")
        dbg_log = nc.dram_tensor("dbg_log", [P, S * H], F32,
                                 kind="ExternalOutput")
        dbg_attn = nc.dram_tensor("dbg_attn", [P, S * H], BF16,
                                  kind="ExternalOutput")
        dbg_av = nc.dram_tensor("dbg_av", [P, C], BF16,
                                kind="ExternalOutput")

    tblp = tbl[:, :].rearrange("(a b) c -> a (b c)", b=2)  # [NPAIR, PROW]

    def bcast_vec(v, cols):
        return bass.AP(tensor=v.ap().tensor, offset=0, ap=[[0, P], [1, cols]])

    with tile.TileContext(nc) as tc:
        import contextlib

        with contextlib.ExitStack() as ctx:
            consts = ctx.enter_context(tc.tile_pool(name="consts", bufs=1))

            nc.gpsimd.load_library(library_config.mlp)

            wq_sb = consts.tile([P, 2, C], BF16)
            wkv_sb = consts.tile([P, 2, 2 * C], BF16)
            wo_sb = consts.tile([P, 2, C], BF16)
            for t_sb, t_dr in ((wq_sb, wq), (wkv_sb, wkv), (wo_sb, wo)):
                nc.sync.dma_start(out=t_sb[:], in_=t_dr[:, :, :])
            w1_sb = consts.tile([4 * S, S * 64], BF16)
            nc.sync.dma_start(out=w1_sb[:], in_=w1b[:, :])
            w2_sb = consts.tile([P, 2 * H], BF16)
            nc.sync.dma_start(out=w2_sb[:], in_=w2[:, :])
            idxr_sb = consts.tile([P, T * NSI], I32)
            nc.sync.dma_start(out=idxr_sb[:], in_=idxr[:, :])
            idx16_sb = consts.tile([P, T * NCH * 32], I16)
            nc.sync.dma_start(out=idx16_sb[:], in_=idx16[:, :])
            mask_sb = consts.tile([P, T * S], F32)
            nc.sync.dma_start(out=mask_sb[:], in_=maskr[:, :])
            relmT_sb = consts.tile([4 * S, T * P], BF16)
            nc.sync.dma_start(out=relmT_sb[:], in_=relmT[:, :])
            bq_sb = consts.tile([P, C], F32)
            nc.sync.dma_start(out=bq_sb[:], in_=bcast_vec(bq, C))
            gam_sb = consts.tile([P, C], F32)
            nc.sync.dma_start(out=gam_sb[:], in_=bcast_vec(gamma, C))
            bet_sb = consts.tile([P, C], F32)
            nc.sync.dma_start(out=bet_sb[:], in_=bcast_vec(beta, C))
            eps_sb = consts.tile([P, 1], F32)
            nc.vector.memset(eps_sb[:], EPS)
            ident = consts.tile([P, P], F32)
            make_identity(nc, ident[:])
            ident_bf = consts.tile([P, P], BF16)
            nc.vector.tensor_copy(out=ident_bf[:], in_=ident[:])

            # ---------------- phase A: neighbor table [k | v] cols 0..511
            # Two g-tiles per iteration; xt loads on the SP HWDGE ring,
            # table stores on the ACT HWDGE ring so the two streams overlap.
            # dedicated rings: loads alternate SP/ACT HWDGE, stores go
            # through SWDGE — no ring carries both (FIFO head-of-line).
            load_engs = (nc.sync, nc.scalar)
            with tc.tile_pool(name="pa", bufs=6) as pa, \
                    tc.tile_pool(name="paps", bufs=2, space="PSUM") as paps:
                for g4 in range(G // 4):
                    xt = pa.tile([P, 4, 2, P], BF16, tag="xt")  # [p,u,j,m]
                    if not ab_Anoload:
                        load_engs[g4 % 2].dma_start(
                            out=xt[:], in_=xtt2[g4, :, :, :, :])
                    row = pa.tile([P, 4, 2 * C], BF16, tag="row")
                    if not ab_Anomm:
                        kvps = paps.tile([P, 4, 2 * C], F32, tag="kvps")
                        for u in range(4):
                            for j in range(2):
                                nc.tensor.matmul(kvps[:, u, :],
                                                 lhsT=xt[:, u, j, :],
                                                 rhs=wkv_sb[:, j, :],
                                                 start=(j == 0),
                                                 stop=(j == 1))
                            if u % 2 == 0:
                                nc.scalar.copy(out=row[:, u, :],
                                               in_=kvps[:, u, :])
                            else:
                                nc.vector.tensor_copy(out=row[:, u, :],
                                                      in_=kvps[:, u, :])
                    # dst rows r = u*128 + p; iterate (p, u, e)
                    if not ab_Anostore:
                        dst = bass.AP(
                            tensor=tbl.ap().tensor,
                            offset=4 * g4 * P * SROW,
                            ap=[[SROW, P], [P * SROW, 4], [1, 2 * C]])
                        nc.gpsimd.dma_start(out=dst, in_=row[:])

            # ---------------- phase B: local attention on own shard
            with contextlib.ExitStack() as bctx:
                pb = bctx.enter_context(tc.tile_pool(name="pb", bufs=2))
                pkv = bctx.enter_context(tc.tile_pool(name="pkv", bufs=2))
                ps_mm = bctx.enter_context(
                    tc.tile_pool(name="ps_mm", bufs=2, space="PSUM"))
                ps_tp = bctx.enter_context(
                    tc.tile_pool(name="ps_tp", bufs=2, space="PSUM"))
                ps_h = bctx.enter_context(
                    tc.tile_pool(name="ps_h", bufs=2, space="PSUM"))
                ps_b = bctx.enter_context(
                    tc.tile_pool(name="ps_b", bufs=1, space="PSUM"))

                for t in range(0 if ab_nophaseB else T):
                    # --- gathers: singles (indirect ring) + pairs (tq ring)
                    sg = pkv.tile([P, NSI, SROW], BF16, tag="sg")
                    pg = pkv.tile([P, NSP, PROW], BF16, tag="pg")
                    if ab_nogather:
                        # same bytes, sequential affine loads (timing ablation)
                        for k in range(NSI):
                            nc.sync.dma_start(
                                out=sg[:, k, :],
                                in_=tbl[t * P:(t + 1) * P, :])
                        for j in range(NSP):
                            nc.sync.dma_start(
                                out=pg[:, j, :],
                                in_=tblp[t * P:t * P + P, :])
                    else:
                        for j in range(NCH):
                            nc.gpsimd.dma_gather(
                                out_ap=pg[:, 4 * j:4 * (j + 1), :],
                                in_ap=tblp,
                                idxs_ap=idx16_sb[:, (t * NCH + j) * 32:
                                                 (t * NCH + j + 1) * 32],
                                num_idxs=512,
                                num_idxs_reg=512,
                                elem_size=PROW,
                            )
                        for k in range(NSI):
                            nc.gpsimd.indirect_dma_start(
                                out=sg[:, k, :], out_offset=None,
                                in_=tbl[:, :],
                                in_offset=IndirectOffsetOnAxis(
                                    ap=idxr_sb[:, t * NSI + k:
                                               t * NSI + k + 1],
                                    axis=0))

                    if ab_nocompute:
                        xq = pb.tile([P, 2, P], BF16, tag="xq")
                        nc.sync.dma_start(out=xq[:], in_=xqt[t, :, :, :])
                        xr = pb.tile([P, C], F32, tag="xr")
                        nc.sync.dma_start(out=xr[:],
                                          in_=xres[t * P:(t + 1) * P, :])
                        yt = pb.tile([P, C], F32, tag="yt")
                        nc.scalar.copy(out=yt[:], in_=xr[:])
                        nc.sync.dma_start(out=y[t * P:(t + 1) * P, :],
                                          in_=yt[:])
                        continue

                    # --- Q projection (pre-scaled)
                    xq = pb.tile([P, 2, P], BF16, tag="xq")
                    nc.sync.dma_start(out=xq[:], in_=xqt[t, :, :, :])
                    qps = ps_mm.tile([P, C], F32, tag="mm")
                    for j in range(2):
                        nc.tensor.matmul(qps[:], lhsT=xq[:, j, :],
                                         rhs=wq_sb[:, j, :],
                                         start=(j == 0), stop=(j == 1))
                    q_sb = pb.tile([P, C], BF16, tag="q")
                    nc.vector.tensor_tensor(out=q_sb[:], in0=qps[:],
                                            in1=bq_sb[:],
                                            op=mybir.AluOpType.add)

                    # --- rel MLP: hid = gelu([rel|1] @ [W1;b1]); relT is a
                    # host-precomputed const slice (pure pos/idx function)
                    relT = relmT_sb[:, t * P:(t + 1) * P]
                    hid_sb = pb.tile([P, S * 64], BF16, tag="hid_sb")
                    for j in range(3):
                        hid_ps = ps_h.tile([P, 512], F32, tag="hid")
                        nc.tensor.matmul(hid_ps[:],
                                         lhsT=relT,
                                         rhs=w1_sb[:, j * 512:(j + 1) * 512],
                                         start=True, stop=True)
                        nc.scalar.activation(
                            out=hid_sb[:, j * 512:(j + 1) * 512],
                            in_=hid_ps[:],
                            func=mybir.ActivationFunctionType.Gelu)
                    nblk = S // 2
                    hidT = pb.tile([P, nblk, P], BF16, tag="hidT")
                    for hh in range(2):
                        hidT_ps = ps_tp.tile([P, nblk // 2, P], BF16,
                                             tag="tp")
                        for b in range(nblk // 2):
                            bb = hh * (nblk // 2) + b
                            nc.tensor.transpose(
                                out=hidT_ps[:, b, :],
                                in_=hid_sb[:, bb * P:(bb + 1) * P],
                                identity=ident_bf[:])
                        nc.scalar.copy(
                            out=hidT[:, hh * (nblk // 2):(hh + 1) * (nblk // 2), :],
                            in_=hidT_ps[:])
                    bias_ps = ps_b.tile([P, S, H], F32, tag="bias")
                    for b in range(nblk):
                        nc.tensor.matmul(
                            bias_ps[:, 2 * b:2 * b + 2, :].rearrange(
                                "p a h -> p (a h)"),
                            lhsT=hidT[:, b, :], rhs=w2_sb[:],
                            start=True, stop=True)

                    # --- logits = q . k_nb  (+ mlp bias + mask), (s, h)
                    prod = pkv.tile([P, S, H, D], BF16, tag="prod")
                    kf_s = bass.AP(tensor=sg.tensor, offset=sg[:].offset,
                                   ap=[sg[:].ap[0], [SROW, NSI], [D, H],
                                       [1, D]])
                    qb_s = bass.AP(tensor=q_sb.tensor, offset=q_sb[:].offset,
                                   ap=[q_sb[:].ap[0], [0, NSI], [D, H],
                                       [1, D]])
                    nc.vector.tensor_tensor(out=prod[:, 0:NSI], in0=kf_s,
                                            in1=qb_s,
                                            op=mybir.AluOpType.mult)
                    kf_p = bass.AP(tensor=pg.tensor, offset=pg[:].offset,
                                   ap=[pg[:].ap[0], [SROW, 2 * NSP],
                                       [D, H], [1, D]])
                    qb_p = bass.AP(tensor=q_sb.tensor, offset=q_sb[:].offset,
                                   ap=[q_sb[:].ap[0], [0, 2 * NSP],
                                       [D, H], [1, D]])
                    nc.vector.tensor_tensor(out=prod[:, NSI:S],
                                            in0=kf_p, in1=qb_p,
                                            op=mybir.AluOpType.mult)
                    logits = pb.tile([P, S, H], F32, tag="log")
                    nc.vector.tensor_reduce(out=logits[:], in_=prod[:],
                                            axis=mybir.AxisListType.X,
                                            op=mybir.AluOpType.add)
                    nc.vector.tensor_tensor(out=logits[:], in0=logits[:],
                                            in1=bias_ps[:],
                                            op=mybir.AluOpType.add)
                    msl = mask_sb[:, t * S:(t + 1) * S]
                    mb = bass.AP(tensor=msl.tensor, offset=msl.offset,
                                 ap=[msl.ap[0], [1, S], [0, H]])
                    nc.vector.tensor_tensor(out=logits[:], in0=logits[:],
                                            in1=mb, op=mybir.AluOpType.add)

                    # --- softmax over s (logits bounded; skip max-sub)
                    pex = pb.tile([P, S, H], F32, tag="pex")
                    nc.scalar.activation(out=pex[:], in_=logits[:],
                                         func=mybir.ActivationFunctionType.Exp)
                    ssum = pb.tile([P, H], F32, tag="ssum")
                    pex_hs = bass.AP(tensor=pex.tensor, offset=pex[:].offset,
                                     ap=[pex[:].ap[0], [1, H], [H, S]])
                    nc.vector.tensor_reduce(out=ssum[:], in_=pex_hs,
                                            axis=mybir.AxisListType.X,
                                            op=mybir.AluOpType.add)
                    rinv = pb.tile([P, H], F32, tag="rinv")
                    nc.vector.reciprocal(out=rinv[:], in_=ssum[:])
                    attn = pb.tile([P, S, H], BF16, tag="attn")
                    rib = bass.AP(tensor=rinv.tensor, offset=rinv[:].offset,
                                  ap=[rinv[:].ap[0], [0, S], [1, H]])
                    nc.vector.tensor_tensor(out=attn[:], in0=pex[:], in1=rib,
                                            op=mybir.AluOpType.mult)

                    # --- av[s] = attn[s] * v[s]; then tree-sum over s
                    av = pkv.tile([P, S, C], BF16, tag="av")
                    vf_s = bass.AP(tensor=sg.tensor,
                                   offset=sg[:].offset + C,
                                   ap=[sg[:].ap[0], [SROW, NSI], [D, H],
                                       [1, D]])
                    at_s = bass.AP(tensor=attn.tensor, offset=attn[:].offset,
                                   ap=[attn[:].ap[0], [H, NSI], [1, H],
                                       [0, D]])
                    nc.vector.tensor_tensor(
                        out=av[:, 0:NSI].rearrange("p s (h d) -> p s h d",
                                                   h=H),
                        in0=vf_s, in1=at_s, op=mybir.AluOpType.mult)
                    vf_p = bass.AP(tensor=pg.tensor,
                                   offset=pg[:].offset + C,
                                   ap=[pg[:].ap[0], [SROW, 2 * NSP],
                                       [D, H], [1, D]])
                    at_p = bass.AP(tensor=attn.tensor,
                                   offset=attn[:].offset + NSI * H,
                                   ap=[attn[:].ap[0], [H, 2 * NSP],
                                       [1, H], [0, D]])
                    nc.vector.tensor_tensor(
                        out=av[:, NSI:S].rearrange("p s (h d) -> p s h d",
                                                   h=H),
                        in0=vf_p, in1=at_p, op=mybir.AluOpType.mult)
                    half = S // 2
                    while half >= 3:
                        nc.vector.tensor_tensor(
                            out=av[:, 0:half], in0=av[:, 0:half],
                            in1=av[:, half:2 * half],
                            op=mybir.AluOpType.add)
                        half //= 2
                    nc.vector.tensor_tensor(out=av[:, 0:1], in0=av[:, 0:1],
                                            in1=av[:, 1:2],
                                            op=mybir.AluOpType.add)
                    nc.vector.tensor_tensor(out=av[:, 0:1], in0=av[:, 0:1],
                                            in1=av[:, 2:3],
                                            op=mybir.AluOpType.add)

                    # --- O projection
                    oT_ps = ps_tp.tile([P, 2, P], BF16, tag="tp")
                    for j in range(2):
                        nc.tensor.transpose(out=oT_ps[:, j, :],
                                            in_=av[:, 0, j * P:(j + 1) * P],
                                            identity=ident_bf[:])
                    oT = pb.tile([P, 2, P], BF16, tag="oT")
                    nc.scalar.copy(out=oT[:], in_=oT_ps[:])
                    ops = ps_mm.tile([P, C], F32, tag="mm")
                    for j in range(2):
                        nc.tensor.matmul(ops[:], lhsT=oT[:, j, :],
                                         rhs=wo_sb[:, j, :],
                                         start=(j == 0), stop=(j == 1))

                    # --- residual (+ folded bv@Wo + bo) + layernorm
                    xr = pb.tile([P, C], F32, tag="xr")
                    nc.sync.dma_start(out=xr[:],
                                      in_=xres[t * P:(t + 1) * P, :])
                    y0 = pb.tile([P, C], F32, tag="y0")
                    nc.vector.tensor_tensor(out=y0[:], in0=ops[:], in1=xr[:],
                                            op=mybir.AluOpType.add)
                    bst = pb.tile([P, 6], F32, tag="bst")
                    nc.vector.bn_stats(out=bst[:], in_=y0[:])
                    mv = pb.tile([P, 2], F32, tag="mv")
                    nc.vector.bn_aggr(out=mv[:], in_=bst[:])
                    std = pb.tile([P, 1], F32, tag="std")
                    nc.scalar.activation(out=std[:], in_=mv[:, 1:2],
                                         func=mybir.ActivationFunctionType.Sqrt,
                                         bias=eps_sb[:])
                    rstd = pb.tile([P, 1], F32, tag="rstd")
                    nc.vector.reciprocal(out=rstd[:], in_=std[:])
                    nc.vector.tensor_scalar(out=y0[:], in0=y0[:],
                                            scalar1=mv[:, 0:1],
                                            scalar2=rstd[:],
                                            op0=mybir.AluOpType.subtract,
                                            op1=mybir.AluOpType.mult)
                    yt = pb.tile([P, C], F32, tag="yt")
                    nc.vector.tensor_tensor(out=yt[:], in0=y0[:],
                                            in1=gam_sb[:],
                                            op=mybir.AluOpType.mult)
                    nc.vector.tensor_tensor(out=yt[:], in0=yt[:],
                                            in1=bet_sb[:],
                                            op=mybir.AluOpType.add)
                    nc.scalar.dma_start(out=y[t * P:(t + 1) * P, :],
                                        in_=yt[:])
                    if dbg and t == 0:
                        nc.sync.dma_start(out=dbg_sg[:, :], in_=sg[:].rearrange(
                            "p a b -> p (a b)"))
                        nc.sync.dma_start(out=dbg_pg[:, :], in_=pg[:].rearrange(
                            "p a b -> p (a b)"))
                        nc.sync.dma_start(out=dbg_q[:, :], in_=q_sb[:])
                        nc.sync.dma_start(out=dbg_hid[:, :], in_=hid_sb[:])
                        nc.sync.dma_start(out=dbg_log[:, :], in_=logits[:].rearrange(
                            "p a b -> p (a b)"))
                        nc.sync.dma_start(out=dbg_attn[:, :], in_=attn[:].rearrange(
                            "p a b -> p (a b)"))
                        nc.sync.dma_start(out=dbg_av[:, :], in_=av[:, 0, :])

    nc.compile()
    return nc


# ------------------------------------------------------------------ driver

_NC_CACHE = {}
RUN_KWARGS = {}
LAST_RESULT = None


def _get_nc(cfg_key, cfg):
    if cfg_key not in _NC_CACHE:
        _NC_CACHE[cfg_key] = build_nc(cfg)
    return _NC_CACHE[cfg_key]


def kernel(**inputs):
    global LAST_RESULT
    from concourse.bass_utils import run_bass_kernel_spmd

    cfg = full_cfg()
    in_maps = host_prep(inputs, cfg)
    nc = _get_nc("full", cfg)
    res = run_bass_kernel_spmd(nc, in_maps, core_ids=list(range(NCORES)),
                               **RUN_KWARGS)
    LAST_RESULT = res
    y = np.concatenate([res.results[c]["y"] for c in range(NCORES)], 0)
    return np.ascontiguousarray(y[:N])


if __name__ == "__main__":
    pass
